# revision 20
# baseline (speedup 1.0000x reference)
"""GRU trajectory decoder on 8 Trainium2 NeuronCores.

Strategy: 8-way tensor parallelism on the hidden dimension (each core owns 128
of the 1024 hidden units of every layer), NOT the data-parallel hint — the
~113MB of GRU weights cannot stay SBUF-resident under data parallelism and
would have to be re-streamed from HBM every timestep (~3.4GB/core). Sharded,
each core holds ~7MB of bf16 weights resident for all 30 steps.

Everything lives in transposed [feature, batch] layout so the per-layer
AllGather (each core contributes its 128 freshly-computed hidden units)
concatenates on the partition axis into exactly the K-major operand the next
matmul needs. The batch is split into two micro-batches of 256 that ping-pong:
while one micro-batch's AllGather is in flight, the other's matmuls run.
bf16 matmul inputs, fp32 PSUM accumulation + elementwise + carried state.
"""
import numpy as np
import ml_dtypes

import concourse.bass as bass
import concourse.mybir as mybir
import concourse.tile as tile
from concourse import bacc
from concourse.bass_utils import run_bass_kernel_spmd

R = 8          # cores / TP degree
P = 128        # partitions; hidden units per core
B = 512        # batch
NB = 256       # micro-batch (2 micro-batches ping-pong)
NUB = B // NB
H = 1024
L = 5
T = 30
ITEM = 4
ZD = 512
GC = 3 * P     # gate columns per core (r,z,n x 128 units)
KT = H // P    # 8 k-tiles over the hidden dim

f32 = mybir.dt.float32
bf16 = mybir.dt.bfloat16
BF = ml_dtypes.bfloat16
AF = mybir.ActivationFunctionType
ALU = mybir.AluOpType

LAST_RESULTS = None  # BassKernelResults of the most recent run (for test.py)
_NC_CACHE = []


def _ktile(w, kt):
    """[kt*128, N] -> [128, kt*N] SBUF layout: out[p, k*N+j] = w[k*128+p, j]."""
    k128, n = w.shape
    assert k128 == kt * 128
    return np.ascontiguousarray(w.reshape(kt, 128, n).transpose(1, 0, 2).reshape(128, kt * n))


def _build():
    nc = bacc.Bacc("TRN2", target_bir_lowering=False, debug=False, num_devices=R)

    # ---- I/O declarations (per-core tensors; data differs per core) ----
    di = {}
    di["wih0"] = nc.dram_tensor("wih0", [ITEM, GC], bf16, kind="ExternalInput")
    for l in range(1, L):
        di[f"wih{l}"] = nc.dram_tensor(f"wih{l}", [P, KT * GC], bf16, kind="ExternalInput")
    for l in range(L):
        di[f"whh{l}"] = nc.dram_tensor(f"whh{l}", [P, KT * GC], bf16, kind="ExternalInput")
        di[f"bias{l}"] = nc.dram_tensor(f"bias{l}", [P, 5], f32, kind="ExternalInput")
    di["w1"] = nc.dram_tensor("w1", [P, 4 * 256], bf16, kind="ExternalInput")
    di["b1"] = nc.dram_tensor("b1", [P, 2], f32, kind="ExternalInput")
    di["w2"] = nc.dram_tensor("w2", [P, 2 * H], bf16, kind="ExternalInput")
    di["b2"] = nc.dram_tensor("b2", [P, KT], f32, kind="ExternalInput")
    di["w2c"] = nc.dram_tensor("w2c", [P, 2 * P], bf16, kind="ExternalInput")
    di["b2c"] = nc.dram_tensor("b2c", [P, 1], f32, kind="ExternalInput")
    di["outw"] = nc.dram_tensor("outw", [P, KT * ITEM], bf16, kind="ExternalInput")
    di["outb"] = nc.dram_tensor("outb", [ITEM, 1], f32, kind="ExternalInput")
    di["zt"] = nc.dram_tensor("zt", [P, 4 * B], bf16, kind="ExternalInput")
    ys = nc.dram_tensor("ys", [T, ITEM, B], f32, kind="ExternalOutput")

    rg = [list(range(R))]

    with tile.TileContext(nc) as tc:
        with (
            tc.tile_pool(name="wsb", bufs=1) as wsb,      # persistent weights
            tc.tile_pool(name="state", bufs=1) as state,  # persistent state
            tc.tile_pool(name="work", bufs=3) as work,    # elementwise temps
            tc.tile_pool(name="psum", bufs=1, space="PSUM") as psum,
            tc.tile_pool(name="dma_in", bufs=4, space="DRAM") as dram_in,
            tc.tile_pool(name="dma_out", bufs=4, space="DRAM") as dram_out,
        ):
            # ---- load weights to SBUF ----
            sb = {}
            for name, t in di.items():
                sb[name] = wsb.tile(list(t.shape), t.dtype, tag=name, name=f"sb_{name}")
                nc.sync.dma_start(sb[name][:], t[:])

            # persistent state tiles
            h_loc = [state.tile([P, B], f32, tag=f"h{l}", name=f"h_loc{l}") for l in range(L)]
            h0_loc = state.tile([P, B], f32, tag="h0loc", name="h0_loc")
            h0_bf = state.tile([P, KT, B], bf16, tag="h0bf", name="h0_bf")
            gath = [[state.tile([P, KT, NB], bf16, tag=f"g{l}_{u}", name=f"gath{l}_{u}")
                     for u in range(NUB)] for l in range(L)]

            def mm(ps, lhsT, rhs, start, stop):
                nc.tensor.matmul(ps, lhsT, rhs, start=start, stop=stop)

            # ---- join MLP (replicated on every core; one-time) ----
            jbf = []
            for m in range(2):
                ps = psum.tile([P, B], f32, tag="g_rz", bufs=3, name="ps_j")
                for k in range(4):
                    mm(ps[:], sb["w1"][:, k * 256 + m * P: k * 256 + (m + 1) * P],
                       sb["zt"][:, k * B:(k + 1) * B], k == 0, k == 3)
                jt = work.tile([P, B], bf16, tag="jbf", name=f"jbf{m}")
                nc.scalar.activation(jt[:], ps[:], AF.Gelu, bias=sb["b1"][:, m:m + 1])
                jbf.append(jt)
            # full h0 (bf16) for step-0 hh matmuls
            for m8 in range(KT):
                ps = psum.tile([P, B], f32, tag="g_n", bufs=3, name="ps_h0")
                for k in range(2):
                    mm(ps[:], sb["w2"][:, k * H + m8 * P: k * H + (m8 + 1) * P],
                       jbf[k][:], k == 0, k == 1)
                nc.scalar.activation(h0_bf[:, m8, :], ps[:], AF.Identity,
                                     bias=sb["b2"][:, m8:m8 + 1])
            # this core's own 128-unit slice of h0 in fp32
            ps = psum.tile([P, B], f32, tag="g_rz", bufs=3, name="ps_h0c")
            for k in range(2):
                mm(ps[:], sb["w2c"][:, k * P:(k + 1) * P], jbf[k][:], k == 0, k == 1)
            nc.scalar.activation(h0_loc[:], ps[:], AF.Identity, bias=sb["b2c"][:, 0:1])

            # step_in = 0
            step_f = []
            step_b = []
            for u in range(NUB):
                sf = work.tile([ITEM, NB], f32, tag="stepf", name=f"step_f{u}")
                nc.vector.memset(sf[:], 0.0)
                sbf = work.tile([ITEM, NB], bf16, tag="stepb", name=f"step_b{u}")
                nc.vector.memset(sbf[:], 0.0)
                step_f.append(sf)
                step_b.append(sbf)

            # ---- the 30 x 5 x 2 cell loop ----
            def emit_warm(n_mm):
                """Dependency-free filler matmuls into a scratch PSUM bank.
                Placed directly before gather-dependent stalls: they bridge the
                PE activity gap so HAM keeps the clock at 2.4 GHz."""
                ps_w = psum.tile([ITEM, NB], f32, tag="g_warm", bufs=1,
                                 name="ps_warm")
                for i in range(n_mm):
                    mm(ps_w[:], sb["outw"][:, 0:ITEM], h0_bf[:, 0, 0:NB],
                       i == 0, i == n_mm - 1)

            def emit_hh(t, l, u):
                """hh matmuls: depend only on last step's gather for this layer,
                so the in-order PE can run them while AllGathers are in flight."""
                ub0, ub1 = u * NB, (u + 1) * NB
                ps_rz = psum.tile([P, 2 * NB], f32, tag="g_rz", bufs=3, name="ps_rz")
                ps_n = psum.tile([P, 2 * NB], f32, tag="g_n", bufs=3, name="ps_n")
                hsrc = (lambda k: h0_bf[:, k, ub0:ub1]) if t == 0 else \
                       (lambda k: gath[l][u][:, k, :])
                whh = sb[f"whh{l}"]
                # start=True clears has_written for the whole 2KB PSUM
                # zero-region (= the whole bank), so each bank gets exactly
                # ONE start; writes to untouched elements overwrite.
                for k in range(KT):
                    for m in range(2):  # r, z
                        mm(ps_rz[:, m * NB:(m + 1) * NB],
                           whh[:, k * GC + m * P: k * GC + (m + 1) * P],
                           hsrc(k), m == 0 and k == 0, False)
                    mm(ps_n[:, 0:NB],
                       whh[:, k * GC + 2 * P: k * GC + 3 * P],
                       hsrc(k), k == 0, False)
                return ps_rz, ps_n

            def emit_ih_elem(t, l, u, ps_rz, ps_n):
                ub0, ub1 = u * NB, (u + 1) * NB
                emit_warm(4)
                # ih matmuls: consume the freshly gathered x of the layer below,
                # k-major so the first chunk's arrival unblocks work early
                if l == 0:
                    wih = sb["wih0"]
                    for m in range(2):
                        mm(ps_rz[:, m * NB:(m + 1) * NB],
                           wih[:, m * P:(m + 1) * P], step_b[u][:], False, m == 1)
                    mm(ps_n[:, NB:2 * NB], wih[:, 2 * P:3 * P], step_b[u][:],
                       False, True)
                else:
                    wih = sb[f"wih{l}"]
                    xsrc = gath[l - 1][u]
                    for k in range(KT):
                        for m in range(2):
                            mm(ps_rz[:, m * NB:(m + 1) * NB],
                               wih[:, k * GC + m * P: k * GC + (m + 1) * P],
                               xsrc[:, k, :], False, m == 1 and k == KT - 1)
                        mm(ps_n[:, NB:2 * NB],
                           wih[:, k * GC + 2 * P: k * GC + 3 * P],
                           xsrc[:, k, :], False, k == KT - 1)

                # elementwise GRU cell on [128, 256] tiles
                bias = sb[f"bias{l}"]
                r_t = work.tile([P, NB], f32, tag="r", name="r_t")
                nc.scalar.activation(r_t[:], ps_rz[:, 0:NB], AF.Sigmoid,
                                     bias=bias[:, 0:1])
                z_t = work.tile([P, NB], f32, tag="z", name="z_t")
                nc.scalar.activation(z_t[:], ps_rz[:, NB:2 * NB], AF.Sigmoid,
                                     bias=bias[:, 1:2])
                rn_t = work.tile([P, NB], f32, tag="rn", name="rn_t")
                nc.vector.scalar_tensor_tensor(
                    rn_t[:], ps_n[:, 0:NB], bias[:, 3:4], r_t[:],
                    op0=ALU.add, op1=ALU.mult)
                np_t = work.tile([P, NB], f32, tag="np", name="np_t")
                nc.vector.scalar_tensor_tensor(
                    np_t[:], ps_n[:, NB:2 * NB], bias[:, 2:3], rn_t[:],
                    op0=ALU.add, op1=ALU.add)
                omz_t = work.tile([P, NB], f32, tag="omz", name="omz_t")
                nc.scalar.activation(omz_t[:], ps_rz[:, NB:2 * NB], AF.Sigmoid,
                                     bias=bias[:, 4:5], scale=-1.0)
                hprev = h0_loc if t == 0 else h_loc[l]
                zh_t = work.tile([P, NB], f32, tag="zh", name="zh_t")
                nc.vector.tensor_mul(zh_t[:], z_t[:], hprev[:, ub0:ub1])
                n_t = work.tile([P, NB], f32, tag="n", name="n_t")
                nc.scalar.activation(n_t[:], np_t[:], AF.Tanh)
                uz_t = work.tile([P, NB], f32, tag="uz", name="uz_t")
                nc.vector.tensor_mul(uz_t[:], n_t[:], omz_t[:])
                hbf_t = work.tile([P, NB], bf16, tag="hbf", name="hbf_t")
                nc.vector.tensor_add(hbf_t[:], uz_t[:], zh_t[:])

                # gather this layer's new hidden state across all 8 cores
                # (trigger emitted before the gpsimd h_loc add so the in-order
                # gpsimd queue never delays the collective)
                cin = dram_in.tile([P, NB], bf16, tag="cin", name="cin")
                nc.sync.dma_start(cin[:], hbf_t[:])
                cout = dram_out.tile([R * P, NB], bf16, tag="cout",
                                     addr_space="Shared", name="cout")
                nc.gpsimd.collective_compute(
                    "AllGather", ALU.bypass, replica_groups=rg,
                    ins=[cin.opt()], outs=[cout.opt()])
                nc.vector.tensor_add(h_loc[l][:, ub0:ub1], uz_t[:], zh_t[:])
                # Gather-out DMAs ride the Scalar HWDGE queue (keeps the Sync
                # queue free for the latency-critical cin bounce). Two splits,
                # not four: each DIRECT2D descriptor-gen costs ~0.65us of
                # serial sequencer time, which directly delays when the
                # gathered halves land after the AllGather completes.
                half = KT // 2 * P
                nc.scalar.dma_start(
                    gath[l][u][:, 0:KT // 2, :],
                    cout[0:half, :].rearrange("(k p) n -> p k n", p=P))
                nc.scalar.dma_start(
                    gath[l][u][:, KT // 2:KT, :],
                    cout[half:2 * half, :].rearrange("(k p) n -> p k n", p=P))

            def emit_head(t, u):
                ub0, ub1 = u * NB, (u + 1) * NB
                ps_hd = psum.tile([ITEM, NB], f32, tag="g_hd", bufs=1, name="ps_hd")
                for k in range(KT):
                    mm(ps_hd[:], sb["outw"][:, k * ITEM:(k + 1) * ITEM],
                       gath[L - 1][u][:, k, :], k == 0, k == KT - 1)
                pred_t = work.tile([ITEM, NB], f32, tag="pred", name="pred_t")
                nc.scalar.activation(pred_t[:], ps_hd[:], AF.Tanh,
                                     bias=sb["outb"][:, 0:1])
                cur_t = work.tile([ITEM, NB], f32, tag="stepf", name="cur_t")
                nc.vector.tensor_add(cur_t[:], pred_t[:], step_f[u][:])
                nc.scalar.dma_start(ys[t, :, ub0:ub1], cur_t[:])
                step_f[u] = cur_t
                nb_t = work.tile([ITEM, NB], bf16, tag="stepb", name="nb_t")
                nc.vector.tensor_copy(nb_t[:], cur_t[:])
                step_b[u] = nb_t

            # Unit stream: unit g = (t*L + l)*NUB + u. hh matmuls are emitted
            # LOOKAHEAD units ahead of their consuming ih so the in-order PE
            # always has gather-independent work queued in front of each
            # AllGather-dependent stall point. Heads for step t are emitted
            # inside step t+1's layer-0 slots (their gather has landed by
            # then, so they never stall the queue).
            LOOKAHEAD = 2
            NU = T * L * NUB

            def unit_tlu(g):
                t, r = divmod(g, L * NUB)
                l, u = divmod(r, NUB)
                return t, l, u

            hh_pend = {}
            for g in range(LOOKAHEAD):
                hh_pend[g] = emit_hh(*unit_tlu(g))
            for g in range(NU):
                t, l, u = unit_tlu(g)
                if g + LOOKAHEAD < NU:
                    hh_pend[g + LOOKAHEAD] = emit_hh(*unit_tlu(g + LOOKAHEAD))
                if l == 0 and t > 0:
                    emit_head(t - 1, u)
                emit_ih_elem(t, l, u, *hh_pend.pop(g))
            for u in range(NUB):
                emit_head(T - 1, u)
    nc.compile()
    return nc


def _in_maps(inputs):
    z = np.asarray(inputs["z"], np.float32)
    w_ih = [np.asarray(inputs["w_ih_l0"], np.float32)] + \
           [np.asarray(inputs["w_ih_rest"][i], np.float32) for i in range(L - 1)]
    w_hh = [np.asarray(inputs["w_hh_l0"], np.float32)] + \
           [np.asarray(inputs["w_hh_rest"][i], np.float32) for i in range(L - 1)]
    b_ih = [np.asarray(inputs["b_ih_l0"], np.float32)] + \
           [np.asarray(inputs["b_ih_rest"][i], np.float32) for i in range(L - 1)]
    b_hh = [np.asarray(inputs["b_hh_l0"], np.float32)] + \
           [np.asarray(inputs["b_hh_rest"][i], np.float32) for i in range(L - 1)]
    w1 = np.asarray(inputs["join_w1"], np.float32)
    b1 = np.asarray(inputs["join_b1"], np.float32)
    w2 = np.asarray(inputs["join_w2"], np.float32)
    b2 = np.asarray(inputs["join_b2"], np.float32)
    outw = np.asarray(inputs["out_w"], np.float32)
    outb = np.asarray(inputs["out_b"], np.float32)

    shared = {
        "w1": _ktile(w1, 4).astype(BF),
        "b1": np.ascontiguousarray(b1.reshape(2, P).T),
        "w2": _ktile(w2, 2).astype(BF),
        "b2": np.ascontiguousarray(b2.reshape(KT, P).T),
        "outw": _ktile(outw, KT).astype(BF),
        "outb": outb.reshape(ITEM, 1),
        "zt": _ktile(np.ascontiguousarray(z.T), 4).astype(BF),
    }
    maps = []
    for c in range(R):
        uc = slice(c * P, (c + 1) * P)
        m = dict(shared)
        for l in range(L):
            gsl = lambda w: np.concatenate(
                [w[:, 0 * H + c * P:0 * H + (c + 1) * P],
                 w[:, 1 * H + c * P:1 * H + (c + 1) * P],
                 w[:, 2 * H + c * P:2 * H + (c + 1) * P]], axis=1)
            whh_c = gsl(w_hh[l])
            m[f"whh{l}"] = _ktile(whh_c, KT).astype(BF)
            if l == 0:
                m["wih0"] = gsl(w_ih[0]).astype(BF)
            else:
                m[f"wih{l}"] = _ktile(gsl(w_ih[l]), KT).astype(BF)
            brz = (b_ih[l] + b_hh[l])
            m[f"bias{l}"] = np.stack(
                [brz[0 * H + c * P:0 * H + (c + 1) * P],
                 brz[1 * H + c * P:1 * H + (c + 1) * P],
                 b_ih[l][2 * H + c * P:2 * H + (c + 1) * P],
                 b_hh[l][2 * H + c * P:2 * H + (c + 1) * P],
                 -brz[1 * H + c * P:1 * H + (c + 1) * P]], axis=1)
        m["w2c"] = _ktile(np.ascontiguousarray(w2[:, uc]), 2).astype(BF)
        m["b2c"] = b2[uc].reshape(P, 1)
        maps.append(m)
    return maps


def kernel(**inputs) -> np.ndarray:
    global LAST_RESULTS
    if not _NC_CACHE:
        _NC_CACHE.append(_build())
    nc = _NC_CACHE[0]
    res = run_bass_kernel_spmd(nc, _in_maps(inputs), core_ids=list(range(R)))
    LAST_RESULTS = res
    ys = res.results[0]["ys"]  # [T, ITEM, B]
    return np.ascontiguousarray(ys.transpose(2, 0, 1))  # [B, T, ITEM]


# revision 21
# speedup vs baseline: 1.1424x; 1.1424x over previous
"""GRU trajectory decoder on 8 Trainium2 NeuronCores.

Strategy: 8-way tensor parallelism on the hidden dimension (each core owns 128
of the 1024 hidden units of every layer), NOT the data-parallel hint — the
~113MB of GRU weights cannot stay SBUF-resident under data parallelism and
would have to be re-streamed from HBM every timestep (~3.4GB/core). Sharded,
each core holds ~7MB of bf16 weights resident for all 30 steps.

Everything lives in transposed [feature, batch] layout so the per-layer
AllGather (each core contributes its 128 freshly-computed hidden units)
concatenates on the partition axis into exactly the K-major operand the next
matmul needs. The batch is split into two micro-batches of 256 that ping-pong:
while one micro-batch's AllGather is in flight, the other's matmuls run.
bf16 matmul inputs, fp32 PSUM accumulation + elementwise + carried state.
"""
import numpy as np
import ml_dtypes

import concourse.bass as bass
import concourse.mybir as mybir
import concourse.tile as tile
from concourse import bacc
from concourse.bass_utils import run_bass_kernel_spmd

R = 8          # cores / TP degree
P = 128        # partitions; hidden units per core
B = 512        # batch
NB = 256       # micro-batch (2 micro-batches ping-pong)
NUB = B // NB
H = 1024
L = 5
T = 30
ITEM = 4
ZD = 512
GC = 3 * P     # gate columns per core (r,z,n x 128 units)
KT = H // P    # 8 k-tiles over the hidden dim

f32 = mybir.dt.float32
bf16 = mybir.dt.bfloat16
BF = ml_dtypes.bfloat16
AF = mybir.ActivationFunctionType
ALU = mybir.AluOpType

LAST_RESULTS = None  # BassKernelResults of the most recent run (for test.py)
_NC_CACHE = []


def _ktile(w, kt):
    """[kt*128, N] -> [128, kt*N] SBUF layout: out[p, k*N+j] = w[k*128+p, j]."""
    k128, n = w.shape
    assert k128 == kt * 128
    return np.ascontiguousarray(w.reshape(kt, 128, n).transpose(1, 0, 2).reshape(128, kt * n))


def _build():
    nc = bacc.Bacc("TRN2", target_bir_lowering=False, debug=False, num_devices=R)

    # ---- I/O declarations (per-core tensors; data differs per core) ----
    di = {}
    di["wih0"] = nc.dram_tensor("wih0", [ITEM, GC], bf16, kind="ExternalInput")
    for l in range(1, L):
        di[f"wih{l}"] = nc.dram_tensor(f"wih{l}", [P, KT * GC], bf16, kind="ExternalInput")
    for l in range(L):
        di[f"whh{l}"] = nc.dram_tensor(f"whh{l}", [P, KT * GC], bf16, kind="ExternalInput")
        di[f"bias{l}"] = nc.dram_tensor(f"bias{l}", [P, 5], f32, kind="ExternalInput")
    di["w1"] = nc.dram_tensor("w1", [P, 4 * 256], bf16, kind="ExternalInput")
    di["b1"] = nc.dram_tensor("b1", [P, 2], f32, kind="ExternalInput")
    di["w2"] = nc.dram_tensor("w2", [P, 2 * H], bf16, kind="ExternalInput")
    di["b2"] = nc.dram_tensor("b2", [P, KT], f32, kind="ExternalInput")
    di["w2c"] = nc.dram_tensor("w2c", [P, 2 * P], bf16, kind="ExternalInput")
    di["b2c"] = nc.dram_tensor("b2c", [P, 1], f32, kind="ExternalInput")
    di["outw"] = nc.dram_tensor("outw", [P, KT * ITEM], bf16, kind="ExternalInput")
    di["outb"] = nc.dram_tensor("outb", [ITEM, 1], f32, kind="ExternalInput")
    di["zt"] = nc.dram_tensor("zt", [P, 4 * B], bf16, kind="ExternalInput")
    ys = nc.dram_tensor("ys", [T, ITEM, B], f32, kind="ExternalOutput")

    rg = [list(range(R))]

    with tile.TileContext(nc) as tc:
        with (
            tc.tile_pool(name="wsb", bufs=1) as wsb,      # persistent weights
            tc.tile_pool(name="state", bufs=1) as state,  # persistent state
            tc.tile_pool(name="work", bufs=3) as work,    # elementwise temps
            tc.tile_pool(name="psum", bufs=1, space="PSUM") as psum,
            tc.tile_pool(name="dma_in", bufs=4, space="DRAM") as dram_in,
            tc.tile_pool(name="dma_out", bufs=4, space="DRAM") as dram_out,
        ):
            # ---- load weights to SBUF ----
            sb = {}
            for name, t in di.items():
                sb[name] = wsb.tile(list(t.shape), t.dtype, tag=name, name=f"sb_{name}")
                nc.sync.dma_start(sb[name][:], t[:])

            # persistent state tiles
            h_loc = [state.tile([P, B], f32, tag=f"h{l}", name=f"h_loc{l}") for l in range(L)]
            h0_loc = state.tile([P, B], f32, tag="h0loc", name="h0_loc")
            h0_bf = state.tile([P, KT, B], bf16, tag="h0bf", name="h0_bf")
            gath = [[state.tile([P, KT, NB], bf16, tag=f"g{l}_{u}", name=f"gath{l}_{u}")
                     for u in range(NUB)] for l in range(L)]

            def mm(ps, lhsT, rhs, start, stop):
                nc.tensor.matmul(ps, lhsT, rhs, start=start, stop=stop)

            # ---- join MLP (replicated on every core; one-time) ----
            jbf = []
            for m in range(2):
                ps = psum.tile([P, B], f32, tag="g_rz", bufs=3, name="ps_j")
                for k in range(4):
                    mm(ps[:], sb["w1"][:, k * 256 + m * P: k * 256 + (m + 1) * P],
                       sb["zt"][:, k * B:(k + 1) * B], k == 0, k == 3)
                jt = work.tile([P, B], bf16, tag="jbf", name=f"jbf{m}")
                nc.scalar.activation(jt[:], ps[:], AF.Gelu, bias=sb["b1"][:, m:m + 1])
                jbf.append(jt)
            # full h0 (bf16) for step-0 hh matmuls
            for m8 in range(KT):
                ps = psum.tile([P, B], f32, tag="g_n", bufs=3, name="ps_h0")
                for k in range(2):
                    mm(ps[:], sb["w2"][:, k * H + m8 * P: k * H + (m8 + 1) * P],
                       jbf[k][:], k == 0, k == 1)
                nc.scalar.activation(h0_bf[:, m8, :], ps[:], AF.Identity,
                                     bias=sb["b2"][:, m8:m8 + 1])
            # this core's own 128-unit slice of h0 in fp32
            ps = psum.tile([P, B], f32, tag="g_rz", bufs=3, name="ps_h0c")
            for k in range(2):
                mm(ps[:], sb["w2c"][:, k * P:(k + 1) * P], jbf[k][:], k == 0, k == 1)
            nc.scalar.activation(h0_loc[:], ps[:], AF.Identity, bias=sb["b2c"][:, 0:1])

            # step_in = 0
            step_f = []
            step_b = []
            for u in range(NUB):
                sf = work.tile([ITEM, NB], f32, tag="stepf", name=f"step_f{u}")
                nc.vector.memset(sf[:], 0.0)
                sbf = work.tile([ITEM, NB], bf16, tag="stepb", name=f"step_b{u}")
                nc.vector.memset(sbf[:], 0.0)
                step_f.append(sf)
                step_b.append(sbf)

            # ---- the 30 x 5 x 2 cell loop ----
            def emit_warm(n_mm):
                """Dependency-free filler matmuls into a scratch PSUM bank.
                Placed directly before gather-dependent stalls: they bridge the
                PE activity gap so HAM keeps the clock at 2.4 GHz."""
                ps_w = psum.tile([ITEM, NB], f32, tag="g_warm", bufs=1,
                                 name="ps_warm")
                for i in range(n_mm):
                    mm(ps_w[:], sb["outw"][:, 0:ITEM], h0_bf[:, 0, 0:NB],
                       i == 0, i == n_mm - 1)

            def emit_hh(t, l, u):
                """hh matmuls: depend only on last step's gather for this layer,
                so the in-order PE can run them while AllGathers are in flight."""
                ub0, ub1 = u * NB, (u + 1) * NB
                ps_rz = psum.tile([P, 2 * NB], f32, tag="g_rz", bufs=3, name="ps_rz")
                ps_n = psum.tile([P, 2 * NB], f32, tag="g_n", bufs=3, name="ps_n")
                hsrc = (lambda k: h0_bf[:, k, ub0:ub1]) if t == 0 else \
                       (lambda k: gath[l][u][:, k, :])
                whh = sb[f"whh{l}"]
                # start=True clears has_written for the whole 2KB PSUM
                # zero-region (= the whole bank), so each bank gets exactly
                # ONE start; writes to untouched elements overwrite.
                for k in range(KT):
                    for m in range(2):  # r, z
                        mm(ps_rz[:, m * NB:(m + 1) * NB],
                           whh[:, k * GC + m * P: k * GC + (m + 1) * P],
                           hsrc(k), m == 0 and k == 0, False)
                    mm(ps_n[:, 0:NB],
                       whh[:, k * GC + 2 * P: k * GC + 3 * P],
                       hsrc(k), k == 0, False)
                return ps_rz, ps_n

            def emit_ih_elem(t, l, u, ps_rz, ps_n):
                ub0, ub1 = u * NB, (u + 1) * NB
                emit_warm(4)
                # ih matmuls: consume the freshly gathered x of the layer below,
                # k-major so the first chunk's arrival unblocks work early
                if l == 0:
                    wih = sb["wih0"]
                    for m in range(2):
                        mm(ps_rz[:, m * NB:(m + 1) * NB],
                           wih[:, m * P:(m + 1) * P], step_b[u][:], False, m == 1)
                    mm(ps_n[:, NB:2 * NB], wih[:, 2 * P:3 * P], step_b[u][:],
                       False, True)
                else:
                    wih = sb[f"wih{l}"]
                    xsrc = gath[l - 1][u]
                    for k in range(KT):
                        for m in range(2):
                            mm(ps_rz[:, m * NB:(m + 1) * NB],
                               wih[:, k * GC + m * P: k * GC + (m + 1) * P],
                               xsrc[:, k, :], False, m == 1 and k == KT - 1)
                        mm(ps_n[:, NB:2 * NB],
                           wih[:, k * GC + 2 * P: k * GC + 3 * P],
                           xsrc[:, k, :], False, k == KT - 1)

                # elementwise GRU cell on [128, 256] tiles
                bias = sb[f"bias{l}"]
                r_t = work.tile([P, NB], f32, tag="r", name="r_t")
                nc.scalar.activation(r_t[:], ps_rz[:, 0:NB], AF.Sigmoid,
                                     bias=bias[:, 0:1])
                z_t = work.tile([P, NB], f32, tag="z", name="z_t")
                nc.scalar.activation(z_t[:], ps_rz[:, NB:2 * NB], AF.Sigmoid,
                                     bias=bias[:, 1:2])
                rn_t = work.tile([P, NB], f32, tag="rn", name="rn_t")
                nc.vector.scalar_tensor_tensor(
                    rn_t[:], ps_n[:, 0:NB], bias[:, 3:4], r_t[:],
                    op0=ALU.add, op1=ALU.mult)
                np_t = work.tile([P, NB], f32, tag="np", name="np_t")
                nc.vector.scalar_tensor_tensor(
                    np_t[:], ps_n[:, NB:2 * NB], bias[:, 2:3], rn_t[:],
                    op0=ALU.add, op1=ALU.add)
                omz_t = work.tile([P, NB], f32, tag="omz", name="omz_t")
                nc.scalar.activation(omz_t[:], ps_rz[:, NB:2 * NB], AF.Sigmoid,
                                     bias=bias[:, 4:5], scale=-1.0)
                hprev = h0_loc if t == 0 else h_loc[l]
                zh_t = work.tile([P, NB], f32, tag="zh", name="zh_t")
                nc.vector.tensor_mul(zh_t[:], z_t[:], hprev[:, ub0:ub1])
                n_t = work.tile([P, NB], f32, tag="n", name="n_t")
                nc.scalar.activation(n_t[:], np_t[:], AF.Tanh)
                uz_t = work.tile([P, NB], f32, tag="uz", name="uz_t")
                nc.vector.tensor_mul(uz_t[:], n_t[:], omz_t[:])
                hbf_t = work.tile([P, NB], bf16, tag="hbf", name="hbf_t")
                nc.vector.tensor_add(hbf_t[:], uz_t[:], zh_t[:])

                # gather this layer's new hidden state across all 8 cores
                # (trigger emitted before the gpsimd h_loc add so the in-order
                # gpsimd queue never delays the collective)
                cin = dram_in.tile([P, NB], bf16, tag="cin", name="cin")
                nc.sync.dma_start(cin[:], hbf_t[:])
                cout = dram_out.tile([R * P, NB], bf16, tag="cout",
                                     addr_space="Shared", name="cout")
                nc.gpsimd.collective_compute(
                    "AllGather", ALU.bypass, replica_groups=rg,
                    ins=[cin.opt()], outs=[cout.opt()])
                nc.vector.tensor_add(h_loc[l][:, ub0:ub1], uz_t[:], zh_t[:])
                # Gather-out DMAs ride the Scalar HWDGE queue (keeps the Sync
                # queue free for the latency-critical cin bounce); quarter
                # splits so the k-major consumers unblock progressively.
                q = KT // 4 * P
                for i in range(4):
                    nc.scalar.dma_start(
                        gath[l][u][:, i * (KT // 4):(i + 1) * (KT // 4), :],
                        cout[i * q:(i + 1) * q, :].rearrange(
                            "(k p) n -> p k n", p=P))

            def emit_head(t, u):
                ub0, ub1 = u * NB, (u + 1) * NB
                ps_hd = psum.tile([ITEM, NB], f32, tag="g_hd", bufs=1, name="ps_hd")
                for k in range(KT):
                    mm(ps_hd[:], sb["outw"][:, k * ITEM:(k + 1) * ITEM],
                       gath[L - 1][u][:, k, :], k == 0, k == KT - 1)
                pred_t = work.tile([ITEM, NB], f32, tag="pred", name="pred_t")
                nc.scalar.activation(pred_t[:], ps_hd[:], AF.Tanh,
                                     bias=sb["outb"][:, 0:1])
                cur_t = work.tile([ITEM, NB], f32, tag="stepf", name="cur_t")
                nc.vector.tensor_add(cur_t[:], pred_t[:], step_f[u][:])
                nc.scalar.dma_start(ys[t, :, ub0:ub1], cur_t[:])
                step_f[u] = cur_t
                nb_t = work.tile([ITEM, NB], bf16, tag="stepb", name="nb_t")
                nc.vector.tensor_copy(nb_t[:], cur_t[:])
                step_b[u] = nb_t

            # Unit stream: unit g = (t*L + l)*NUB + u. hh matmuls are emitted
            # LOOKAHEAD units ahead of their consuming ih so the in-order PE
            # always has gather-independent work queued in front of each
            # AllGather-dependent stall point. Heads for step t are emitted
            # inside step t+1's layer-0 slots (their gather has landed by
            # then, so they never stall the queue).
            LOOKAHEAD = 2
            NU = T * L * NUB

            def unit_tlu(g):
                t, r = divmod(g, L * NUB)
                l, u = divmod(r, NUB)
                return t, l, u

            hh_pend = {}
            for g in range(LOOKAHEAD):
                hh_pend[g] = emit_hh(*unit_tlu(g))
            for g in range(NU):
                t, l, u = unit_tlu(g)
                if g + LOOKAHEAD < NU:
                    hh_pend[g + LOOKAHEAD] = emit_hh(*unit_tlu(g + LOOKAHEAD))
                if l == 0 and t > 0:
                    emit_head(t - 1, u)
                emit_ih_elem(t, l, u, *hh_pend.pop(g))
            for u in range(NUB):
                emit_head(T - 1, u)
    nc.compile()
    return nc


def _in_maps(inputs):
    z = np.asarray(inputs["z"], np.float32)
    w_ih = [np.asarray(inputs["w_ih_l0"], np.float32)] + \
           [np.asarray(inputs["w_ih_rest"][i], np.float32) for i in range(L - 1)]
    w_hh = [np.asarray(inputs["w_hh_l0"], np.float32)] + \
           [np.asarray(inputs["w_hh_rest"][i], np.float32) for i in range(L - 1)]
    b_ih = [np.asarray(inputs["b_ih_l0"], np.float32)] + \
           [np.asarray(inputs["b_ih_rest"][i], np.float32) for i in range(L - 1)]
    b_hh = [np.asarray(inputs["b_hh_l0"], np.float32)] + \
           [np.asarray(inputs["b_hh_rest"][i], np.float32) for i in range(L - 1)]
    w1 = np.asarray(inputs["join_w1"], np.float32)
    b1 = np.asarray(inputs["join_b1"], np.float32)
    w2 = np.asarray(inputs["join_w2"], np.float32)
    b2 = np.asarray(inputs["join_b2"], np.float32)
    outw = np.asarray(inputs["out_w"], np.float32)
    outb = np.asarray(inputs["out_b"], np.float32)

    shared = {
        "w1": _ktile(w1, 4).astype(BF),
        "b1": np.ascontiguousarray(b1.reshape(2, P).T),
        "w2": _ktile(w2, 2).astype(BF),
        "b2": np.ascontiguousarray(b2.reshape(KT, P).T),
        "outw": _ktile(outw, KT).astype(BF),
        "outb": outb.reshape(ITEM, 1),
        "zt": _ktile(np.ascontiguousarray(z.T), 4).astype(BF),
    }
    maps = []
    for c in range(R):
        uc = slice(c * P, (c + 1) * P)
        m = dict(shared)
        for l in range(L):
            gsl = lambda w: np.concatenate(
                [w[:, 0 * H + c * P:0 * H + (c + 1) * P],
                 w[:, 1 * H + c * P:1 * H + (c + 1) * P],
                 w[:, 2 * H + c * P:2 * H + (c + 1) * P]], axis=1)
            whh_c = gsl(w_hh[l])
            m[f"whh{l}"] = _ktile(whh_c, KT).astype(BF)
            if l == 0:
                m["wih0"] = gsl(w_ih[0]).astype(BF)
            else:
                m[f"wih{l}"] = _ktile(gsl(w_ih[l]), KT).astype(BF)
            brz = (b_ih[l] + b_hh[l])
            m[f"bias{l}"] = np.stack(
                [brz[0 * H + c * P:0 * H + (c + 1) * P],
                 brz[1 * H + c * P:1 * H + (c + 1) * P],
                 b_ih[l][2 * H + c * P:2 * H + (c + 1) * P],
                 b_hh[l][2 * H + c * P:2 * H + (c + 1) * P],
                 -brz[1 * H + c * P:1 * H + (c + 1) * P]], axis=1)
        m["w2c"] = _ktile(np.ascontiguousarray(w2[:, uc]), 2).astype(BF)
        m["b2c"] = b2[uc].reshape(P, 1)
        maps.append(m)
    return maps


def kernel(**inputs) -> np.ndarray:
    global LAST_RESULTS
    if not _NC_CACHE:
        _NC_CACHE.append(_build())
    nc = _NC_CACHE[0]
    res = run_bass_kernel_spmd(nc, _in_maps(inputs), core_ids=list(range(R)))
    LAST_RESULTS = res
    ys = res.results[0]["ys"]  # [T, ITEM, B]
    return np.ascontiguousarray(ys.transpose(2, 0, 1))  # [B, T, ITEM]
